# revision 8
# baseline (speedup 1.0000x reference)
"""AttributeDecoupledGNN Trainium2 kernel (8-core SPMD).

Strategy:
  - All node features kept transposed on-chip: [128 feats, node-slots].
  - Nodes dst-sharded: 12500/core, assigned to 13312 "slots" (208 bins x 64)
    via balanced bin-packing so each (bin, src-chunk) has <= 256 edges ->
    exactly 2 gather tiles of 128 edges -> cross-core-uniform program.
  - mean-aggregation = dma_gather (bf16 256B rows, int16 idx, 4 chunks of
    26624 table rows) + PE one-hot S-matmul (fp8 S) into PSUM windows of 512
    slots, accumulated chunk-by-chunk into an SBUF f32 accumulator, then
    scaled by 1/deg.
  - h shards exchanged between layers via AllGather collectives into a
    row-major gather table.
  - dist path + final layer folded: logits = h3 @ (W_np @ fW_a) +
    y3 @ (d_W3 @ fW_b) + const.
"""
import numpy as np
import ml_dtypes

import concourse.bass as bass
import concourse.bacc as bacc
import concourse.tile as tile
import concourse.mybir as mybir
from concourse.bass_utils import run_bass_kernel_spmd
from concourse.masks import make_identity

dt = mybir.dt
P = 128

# ---------------- problem constants (hardcoded) ----------------
N = 100000
E = 1600000
F_IN = 256
H = 128
KATT = 5
NCORES = 8
NSH = N // NCORES              # 12500
SLOTS = 13312                  # 26 windows * 512 = 208 bins * 64
WINDOWS = SLOTS // 512         # 26
BINS = SLOTS // 64             # 208
BIN_COLS = 64
T_S = 2                        # tiles per (bin, chunk)
NCHUNKS = 4
CHUNK_ROWS = 2 * SLOTS         # 26624
TILES_PER_CHUNK = BINS * T_S   # 416
IDX_PER_CHUNK = TILES_PER_CHUNK * 128   # 53248
CALL_TILES = 32                # 2 windows per gather call
CALLS_PER_CHUNK = (TILES_PER_CHUNK + CALL_TILES - 1) // CALL_TILES  # 13
NTAB = NCORES * SLOTS          # 106496
NODE_CHUNK = 512               # nodes per dense-phase matmul


# ================= host preprocessing =================

def _wrap_idx(idxs):
    return idxs.reshape(-1, 16).T.copy()


def _assign_bins(cnt):
    cap = T_S * 128
    fill = np.zeros((BINS, NCHUNKS), dtype=np.int64)
    ncols = np.zeros(BINS, dtype=np.int64)
    order = np.argsort(-cnt.max(axis=1), kind="stable")
    slot = np.full(cnt.shape[0], -1, dtype=np.int64)
    for d in order:
        c = cnt[d]
        new_fill = fill + c[None, :]
        feas = (new_fill <= cap).all(axis=1) & (ncols < BIN_COLS)
        if not feas.any():
            raise RuntimeError("bin packing infeasible")
        score = new_fill.max(axis=1).astype(np.float64)
        score[~feas] = np.inf
        b = int(np.argmin(score + 0.001 * ncols))
        slot[d] = b * BIN_COLS + ncols[b]
        ncols[b] += 1
        fill[b] += c
    return slot


def _preprocess_edges(edge_index):
    src = np.asarray(edge_index[0], dtype=np.int64)
    dst = np.asarray(edge_index[1], dtype=np.int64)

    deg = np.bincount(dst, minlength=N).astype(np.float32)
    recip_node = 1.0 / np.maximum(deg, 1.0)

    dst_owner = dst // NSH
    dst_local = dst % NSH
    src_owner = src // NSH
    chunk = src_owner // 2

    slot_of_node = np.zeros(N, dtype=np.int64)
    core_slotmap = []
    for c in range(NCORES):
        m = dst_owner == c
        cnt = np.zeros((NSH, NCHUNKS), dtype=np.int64)
        np.add.at(cnt, (dst_local[m], chunk[m]), 1)
        slot = _assign_bins(cnt)
        nodes = c * NSH + np.arange(NSH)
        slot_of_node[nodes] = slot
        smap = np.full(SLOTS, -1, dtype=np.int64)
        smap[slot] = nodes
        core_slotmap.append(smap)
    global_row_of_node = (np.arange(N) // NSH) * SLOTS + slot_of_node

    per_core = []
    for c in range(NCORES):
        m = dst_owner == c
        e_src_row = global_row_of_node[src[m]]
        e_slot = slot_of_node[dst[m]]
        e_chunk = e_src_row // CHUNK_ROWS
        e_idx_local = e_src_row % CHUNK_ROWS
        e_bin = e_slot // BIN_COLS
        e_col = e_slot % BIN_COLS

        key = e_chunk * BINS + e_bin
        order = np.argsort(key, kind="stable")
        key_s = key[order]
        idx_s = e_idx_local[order]
        col_s = e_col[order]
        bounds = np.searchsorted(key_s, np.arange(NCHUNKS * BINS + 1))

        idx_stream = np.zeros(NCHUNKS * IDX_PER_CHUNK, dtype=np.int16)
        scol_stream = np.full(NCHUNKS * IDX_PER_CHUNK, -1, dtype=np.int16)
        cap = T_S * 128
        for q in range(NCHUNKS):
            for b in range(BINS):
                k = q * BINS + b
                lo, hi = bounds[k], bounds[k + 1]
                n = hi - lo
                base = q * IDX_PER_CHUNK + b * cap
                idx_stream[base:base + n] = idx_s[lo:hi]
                scol_stream[base:base + n] = col_s[lo:hi]

        idx_wrapped = np.zeros((16, NCHUNKS * IDX_PER_CHUNK // 16), dtype=np.int16)
        off = 0
        for q in range(NCHUNKS):
            for k in range(CALLS_PER_CHUNK):
                t0 = k * CALL_TILES
                t1 = min(t0 + CALL_TILES, TILES_PER_CHUNK)
                nidx = (t1 - t0) * 128
                seg = idx_stream[q * IDX_PER_CHUNK + t0 * 128:
                                 q * IDX_PER_CHUNK + t1 * 128]
                idx_wrapped[:, off:off + nidx // 16] = _wrap_idx(seg)
                off += nidx // 16
        idx_rep = np.zeros((128, NCHUNKS * IDX_PER_CHUNK // 16), dtype=np.int16)
        for g in range(8):
            idx_rep[g * 16:(g + 1) * 16] = idx_wrapped

        ntiles = NCHUNKS * TILES_PER_CHUNK
        S = np.zeros((128, ntiles * BIN_COLS), dtype=ml_dtypes.float8_e4m3)
        scol_t = scol_stream.reshape(ntiles, 128)
        tt, pp = np.nonzero(scol_t >= 0)
        S[pp, tt * BIN_COLS + scol_t[tt, pp]] = 1.0

        smap = core_slotmap[c]
        recip_slot = np.zeros(SLOTS, dtype=np.float32)
        valid = smap >= 0
        recip_slot[valid] = recip_node[smap[valid]]

        per_core.append(dict(idx=idx_rep, S=S,
                             recip=np.broadcast_to(recip_slot[None, :],
                                                   (128, SLOTS)).copy(),
                             slotmap=smap))

    return per_core, global_row_of_node, slot_of_node


# ================= device program =================

def _build_program():
    nc = bacc.Bacc("TRN2", target_bir_lowering=False, debug=False,
                   enable_asserts=False, num_devices=NCORES)

    # per-core inputs
    x_t = nc.dram_tensor("x_t", [2, 128, SLOTS], dt.bfloat16, kind="ExternalInput")
    x_full = nc.dram_tensor("x_full", [2, 128, NTAB], dt.bfloat16, kind="ExternalInput")
    attr_t = nc.dram_tensor("attr_t", [KATT, SLOTS], dt.bfloat16, kind="ExternalInput")
    idx_d = nc.dram_tensor("idx_d", [128, NCHUNKS * IDX_PER_CHUNK // 16], dt.int16,
                           kind="ExternalInput")
    s_d = nc.dram_tensor("s_d", [128, NCHUNKS * TILES_PER_CHUNK * BIN_COLS],
                         dt.float8e4, kind="ExternalInput")
    recip_d = nc.dram_tensor("recip_d", [128, WINDOWS * 512], dt.float32, kind="ExternalInput")
    # replicated weights
    w_pre = nc.dram_tensor("w_pre", [2, 128, H], dt.bfloat16, kind="ExternalInput")
    w_conv = nc.dram_tensor("w_conv", [4, 128, H], dt.bfloat16, kind="ExternalInput")
    w_dist = nc.dram_tensor("w_dist", [2, 128, H], dt.bfloat16, kind="ExternalInput")
    w_d0 = nc.dram_tensor("w_d0", [KATT, H], dt.bfloat16, kind="ExternalInput")
    w_fin = nc.dram_tensor("w_fin", [2, 128, 1], dt.bfloat16, kind="ExternalInput")
    biases = nc.dram_tensor("biases", [128, 8], dt.float32, kind="ExternalInput")
    # biases cols: 0=pre_b 1=c1_b 2=c2_b 3=d_b0 4=d_b1 5=d_b2 6=(c0 scalar in [0,6]) 7=unused

    out_d = nc.dram_tensor("out_d", [1, SLOTS], dt.float32, kind="ExternalOutput")

    AF = mybir.ActivationFunctionType

    with tile.TileContext(nc) as tc:
        with (
            tc.tile_pool(name="res", bufs=1) as res,
            tc.tile_pool(name="sb", bufs=2) as sb,
            tc.tile_pool(name="ps", bufs=2, space="PSUM") as ps,
            tc.tile_pool(name="dram", bufs=1, space="DRAM") as dram,
        ):
            # ---- resident tiles ----
            h_cur = res.tile([128, SLOTS], dt.bfloat16, tag="h_a")    # h1/h3
            h_nxt = res.tile([128, SLOTS], dt.bfloat16, tag="h_b")    # h2
            agg_t = res.tile([128, SLOTS], dt.bfloat16, tag="agg")
            acc = res.tile([128, SLOTS], dt.float32, tag="acc")
            wpre_sb = res.tile([128, 2 * H], dt.bfloat16, tag="wpre")
            wconv_sb = res.tile([128, 4 * H], dt.bfloat16, tag="wconv")
            wdist_sb = res.tile([128, 2 * H], dt.bfloat16, tag="wdist")
            wd0_sb = res.tile([KATT, H], dt.bfloat16, tag="wd0")
            wfin_sb = res.tile([128, 2], dt.bfloat16, tag="wfin")
            bias_sb = res.tile([128, 8], dt.float32, tag="bias")
            ident = res.tile([128, 128], dt.bfloat16, tag="ident")

            nc.sync.dma_start(wpre_sb[:].rearrange("p (k h) -> p k h", k=2), w_pre.ap().rearrange("k p h -> p k h"))
            nc.sync.dma_start(wconv_sb[:].rearrange("p (k h) -> p k h", k=4), w_conv.ap().rearrange("k p h -> p k h"))
            nc.sync.dma_start(wdist_sb[:].rearrange("p (k h) -> p k h", k=2), w_dist.ap().rearrange("k p h -> p k h"))
            nc.sync.dma_start(wd0_sb[:], w_d0[:])
            nc.sync.dma_start(wfin_sb[:].rearrange("p (k o) -> p k o", k=2), w_fin.ap().rearrange("k p o -> p k o"))
            nc.sync.dma_start(bias_sb[:], biases[:])
            make_identity(nc, ident[:])

            # gather tables + exchange bounce (DRAM)
            table1 = dram.tile([NTAB, H], dt.bfloat16, tag="table1")
            table2 = dram.tile([NTAB, H], dt.bfloat16, tag="table2", addr_space="Shared")
            bounce2 = dram.tile([SLOTS, H], dt.bfloat16, tag="bounce2")

            # ---------------- dense helpers ----------------

            def pre_full_phase():
                """full-graph pre-matmul -> row-major table1 (local, no collective)."""
                for j in range(NTAB // NODE_CHUNK):
                    js = slice(j * NODE_CHUNK, (j + 1) * NODE_CHUNK)
                    xs = sb.tile([128, 2, NODE_CHUNK], dt.bfloat16, tag="xstage")
                    nc.sync.dma_start(
                        xs[:], x_full.ap()[:, :, js].rearrange("k p n -> p k n"))
                    pm = ps.tile([128, NODE_CHUNK], dt.float32, space="PSUM", tag="mm")
                    nc.tensor.matmul(pm[:], lhsT=wpre_sb[:, 0:H], rhs=xs[:, 0, :],
                                     start=True, stop=False)
                    nc.tensor.matmul(pm[:], lhsT=wpre_sb[:, H:2 * H], rhs=xs[:, 1, :],
                                     start=False, stop=True)
                    hs = sb.tile([128, NODE_CHUNK], dt.bfloat16, tag="hstage")
                    nc.vector.tensor_add(
                        hs[:], in0=pm[:],
                        in1=bias_sb[:, 0:1].to_broadcast([128, NODE_CHUNK]))
                    rs = sb.tile([128, 4, 128], dt.bfloat16, tag="rowstage")
                    for b in range(4):
                        pt = ps.tile([128, 128], dt.bfloat16, space="PSUM", tag="tr")
                        nc.tensor.transpose(out=pt[:], in_=hs[:, b * 128:(b + 1) * 128],
                                            identity=ident[:])
                        nc.scalar.copy(rs[:, b, :], pt[:])
                    nc.sync.dma_start(
                        table1[j * NODE_CHUNK:(j + 1) * NODE_CHUNK, :]
                        .rearrange("(b p) d -> p b d", p=128),
                        rs[:])

            def pre_phase():
                """h_cur[:, :] = x @ pre_W + pre_b (sharded, transposed)."""
                for j in range(SLOTS // NODE_CHUNK):
                    js = slice(j * NODE_CHUNK, (j + 1) * NODE_CHUNK)
                    xs = sb.tile([128, 2, NODE_CHUNK], dt.bfloat16, tag="xstage")
                    nc.sync.dma_start(
                        xs[:], x_t.ap()[:, :, js].rearrange("k p n -> p k n"))
                    pm = ps.tile([128, NODE_CHUNK], dt.float32, space="PSUM", tag="mm")
                    nc.tensor.matmul(pm[:], lhsT=wpre_sb[:, 0:H], rhs=xs[:, 0, :],
                                     start=True, stop=False)
                    nc.tensor.matmul(pm[:], lhsT=wpre_sb[:, H:2 * H], rhs=xs[:, 1, :],
                                     start=False, stop=True)
                    nc.vector.tensor_add(
                        h_cur[:, js], in0=pm[:],
                        in1=bias_sb[:, 0:1].to_broadcast([128, NODE_CHUNK]))

            def conv_phase(h_in, h_out, w_off, bias_col):
                """h_out = relu(Ws.T h_in + Wn.T agg + b)."""
                for j in range(SLOTS // NODE_CHUNK):
                    js = slice(j * NODE_CHUNK, (j + 1) * NODE_CHUNK)
                    pm = ps.tile([128, NODE_CHUNK], dt.float32, space="PSUM", tag="mm")
                    nc.tensor.matmul(pm[:], lhsT=wconv_sb[:, w_off * H:(w_off + 1) * H],
                                     rhs=h_in[:, js], start=True, stop=False)
                    nc.tensor.matmul(pm[:], lhsT=wconv_sb[:, (w_off + 1) * H:(w_off + 2) * H],
                                     rhs=agg_t[:, js], start=False, stop=True)
                    nc.scalar.activation(h_out[:, js], pm[:], AF.Relu,
                                         bias=bias_sb[:, bias_col:bias_col + 1])

            def exchange(h_shard, bounce, table):
                """transpose shard -> bounce -> AllGather -> table."""
                for j in range(SLOTS // NODE_CHUNK):
                    rs = sb.tile([128, 4, 128], dt.bfloat16, tag="rowstage")
                    for b in range(4):
                        col = j * NODE_CHUNK + b * 128
                        pt = ps.tile([128, 128], dt.bfloat16, space="PSUM", tag="tr")
                        nc.tensor.transpose(out=pt[:], in_=h_shard[:, col:col + 128],
                                            identity=ident[:])
                        nc.scalar.copy(rs[:, b, :], pt[:])
                    nc.sync.dma_start(
                        bounce[j * NODE_CHUNK:(j + 1) * NODE_CHUNK, :]
                        .rearrange("(b p) d -> p b d", p=128),
                        rs[:])
                nc.gpsimd.collective_compute(
                    "AllGather", mybir.AluOpType.bypass,
                    replica_groups=[list(range(NCORES))],
                    ins=[bounce.opt()],
                    outs=[table.opt()],
                )

            def agg_phase(table):
                """acc = segment-sum over edges (gather + S matmul); agg_t = acc * recip."""
                for q in range(NCHUNKS):
                    ih = sb.tile([128, IDX_PER_CHUNK // 16], dt.int16, tag="idxstage")
                    nc.sync.dma_start(
                        ih[:], idx_d[:, q * (IDX_PER_CHUNK // 16):
                                     (q + 1) * (IDX_PER_CHUNK // 16)])
                    SGRP = 32  # tiles per S stage (2 windows)
                    shs = []
                    for g in range(TILES_PER_CHUNK // SGRP):
                        sh = sb.tile([128, SGRP * BIN_COLS], dt.float8e4, tag="sstage")
                        base = (q * TILES_PER_CHUNK + g * SGRP) * BIN_COLS
                        nc.scalar.dma_start(
                            sh[:], s_d[:, base:base + SGRP * BIN_COLS])
                        shs.append(sh)

                    gts = []
                    for k in range(CALLS_PER_CHUNK):
                        t0 = k * CALL_TILES
                        t1 = min(t0 + CALL_TILES, TILES_PER_CHUNK)
                        nidx = (t1 - t0) * 128
                        gt = sb.tile([128, CALL_TILES, H], dt.bfloat16, tag="gbuf")
                        nc.gpsimd.dma_gather(
                            gt[:, 0:(t1 - t0), :],
                            table[q * CHUNK_ROWS:(q + 1) * CHUNK_ROWS, :],
                            ih[:, t0 * 8:t0 * 8 + nidx // 16],
                            nidx, nidx, H, single_packet=False,
                        )
                        gts.append((gt, t0, t1))

                    # consume: per window (8 bins = 16 tiles)
                    for w in range(WINDOWS):
                        pw = ps.tile([128, 512], dt.float32, space="PSUM", tag="aggps")
                        for bi in range(8):
                            b = w * 8 + bi
                            for s_i in range(T_S):
                                t = b * T_S + s_i
                                gt, t0, t1 = gts[t // CALL_TILES]
                                sg = t // 32
                                soff = (t - sg * 32) * BIN_COLS
                                nc.tensor.matmul(
                                    pw[:, bi * BIN_COLS:(bi + 1) * BIN_COLS],
                                    lhsT=gt[:, t - t0, :],
                                    rhs=shs[sg][:, soff:soff + BIN_COLS],
                                    start=(bi == 0 and s_i == 0),
                                    stop=(bi == 7 and s_i == T_S - 1),
                                )
                        ws = slice(w * 512, (w + 1) * 512)
                        if q == 0:
                            nc.vector.tensor_copy(acc[:, ws], pw[:])
                        else:
                            nc.vector.tensor_add(acc[:, ws], in0=acc[:, ws], in1=pw[:])

                # scale by recip -> bf16 agg
                for w in range(WINDOWS):
                    ws = slice(w * 512, (w + 1) * 512)
                    rc = sb.tile([128, 512], dt.float32, tag="recip")
                    nc.sync.dma_start(rc[:], recip_d[:, w * 512:(w + 1) * 512])
                    nc.vector.tensor_mul(agg_t[:, ws], in0=acc[:, ws], in1=rc[:])

            def dist_final_phase(h3):
                """fused dist MLP + folded final layer + sigmoid."""
                for j in range(SLOTS // NODE_CHUNK):
                    js = slice(j * NODE_CHUNK, (j + 1) * NODE_CHUNK)
                    at = sb.tile([KATT, NODE_CHUNK], dt.bfloat16, tag="attrstage")
                    nc.sync.dma_start(at[:], attr_t.ap()[:, js])
                    p1 = ps.tile([128, NODE_CHUNK], dt.float32, space="PSUM", tag="mm")
                    nc.tensor.matmul(p1[:], lhsT=wd0_sb[:], rhs=at[:],
                                     start=True, stop=True)
                    y1 = sb.tile([128, NODE_CHUNK], dt.bfloat16, tag="y1")
                    nc.scalar.activation(y1[:], p1[:], AF.Relu, bias=bias_sb[:, 3:4])
                    p2 = ps.tile([128, NODE_CHUNK], dt.float32, space="PSUM", tag="mm")
                    nc.tensor.matmul(p2[:], lhsT=wdist_sb[:, 0:H], rhs=y1[:],
                                     start=True, stop=True)
                    y2 = sb.tile([128, NODE_CHUNK], dt.bfloat16, tag="y2")
                    nc.scalar.activation(y2[:], p2[:], AF.Relu, bias=bias_sb[:, 4:5])
                    p3 = ps.tile([128, NODE_CHUNK], dt.float32, space="PSUM", tag="mm")
                    nc.tensor.matmul(p3[:], lhsT=wdist_sb[:, H:2 * H], rhs=y2[:],
                                     start=True, stop=True)
                    y3 = sb.tile([128, NODE_CHUNK], dt.bfloat16, tag="y3")
                    nc.scalar.activation(y3[:], p3[:], AF.Relu, bias=bias_sb[:, 5:6])
                    pf = ps.tile([1, NODE_CHUNK], dt.float32, space="PSUM", tag="fin")
                    nc.tensor.matmul(pf[:], lhsT=wfin_sb[:, 0:1], rhs=h3[:, js],
                                     start=True, stop=False)
                    nc.tensor.matmul(pf[:], lhsT=wfin_sb[:, 1:2], rhs=y3[:],
                                     start=False, stop=True)
                    ot = sb.tile([1, NODE_CHUNK], dt.float32, tag="ostage")
                    nc.scalar.activation(ot[:], pf[:], AF.Sigmoid,
                                         bias=bias_sb[0:1, 6:7])
                    nc.sync.dma_start(out_d[:, js], ot[:])

            # ---------------- schedule ----------------
            pre_full_phase()                   # table1 = h1 (all rows, local)
            pre_phase()                        # h_cur = h1 own shard
            agg_phase(table1)                  # agg_t = mean_agg(h1)
            conv_phase(h_cur, h_nxt, 0, 1)     # h_nxt = h2
            exchange(h_nxt, bounce2, table2)   # table2 = h2
            agg_phase(table2)                  # agg_t = mean_agg(h2)
            conv_phase(h_nxt, h_cur, 2, 2)     # h_cur = h3
            dist_final_phase(h_cur)

    nc.compile()
    return nc


_PROGRAM_CACHE = {}


def kernel(**inputs):
    x = np.asarray(inputs["x"], dtype=np.float32)
    edge_index = np.asarray(inputs["edge_index"])
    edge_attr = np.asarray(inputs["edge_attr"], dtype=np.float32)

    per_core, global_row_of_node, slot_of_node = _preprocess_edges(edge_index)

    bf = ml_dtypes.bfloat16
    f32 = np.float32

    pre_W = np.asarray(inputs["pre_W"], f32)
    w_pre = np.ascontiguousarray(pre_W.reshape(2, 128, H)).astype(bf)
    w_conv = np.stack([np.asarray(inputs["c1_Ws"], f32), np.asarray(inputs["c1_Wn"], f32),
                       np.asarray(inputs["c2_Ws"], f32), np.asarray(inputs["c2_Wn"], f32)]
                      ).astype(bf)
    w_dist = np.stack([np.asarray(inputs["d_W1"], f32),
                       np.asarray(inputs["d_W2"], f32)]).astype(bf)
    w_d0 = np.asarray(inputs["d_W0"], f32).astype(bf)

    fW = np.asarray(inputs["final_W"], f32)           # [256, 1]
    w1 = np.asarray(inputs["nodepost_W"], f32) @ fW[:128]   # [128,1]
    w2 = np.asarray(inputs["d_W3"], f32) @ fW[128:]         # [128,1]
    w_fin = np.stack([w1, w2]).astype(bf)                   # [2,128,1]
    c0 = float(np.asarray(inputs["nodepost_b"], f32) @ fW[:128, 0]
               + np.asarray(inputs["d_b3"], f32) @ fW[128:, 0]
               + np.asarray(inputs["final_b"], f32)[0])

    biases = np.zeros((128, 8), f32)
    biases[:, 0] = np.asarray(inputs["pre_b"], f32)
    biases[:, 1] = np.asarray(inputs["c1_b"], f32)
    biases[:, 2] = np.asarray(inputs["c2_b"], f32)
    biases[:, 3] = np.asarray(inputs["d_b0"], f32)
    biases[:, 4] = np.asarray(inputs["d_b1"], f32)
    biases[:, 5] = np.asarray(inputs["d_b2"], f32)
    biases[0, 6] = c0

    if "nc" not in _PROGRAM_CACHE:
        _PROGRAM_CACHE["nc"] = _build_program()
    nc = _PROGRAM_CACHE["nc"]

    x_ts = []
    for c in range(NCORES):
        smap = per_core[c]["slotmap"]
        valid = smap >= 0
        x_tc = np.zeros((2, 128, SLOTS), bf)
        xv = x[smap[valid]].astype(bf)                 # [n_valid, 256]
        x_tc[:, :, :][..., valid] = xv.T.reshape(2, 128, -1)
        x_ts.append(x_tc)
    x_full_np = np.concatenate(x_ts, axis=2)           # [2, 128, NTAB]

    in_maps = []
    for c in range(NCORES):
        pc = per_core[c]
        smap = pc["slotmap"]
        valid = smap >= 0
        attr_t = np.zeros((KATT, SLOTS), bf)
        attr_t[:, valid] = np.asarray(edge_attr, f32)[smap[valid]].T.astype(bf)
        in_maps.append({
            "x_t": x_ts[c], "x_full": x_full_np, "attr_t": attr_t,
            "idx_d": pc["idx"], "s_d": np.asarray(pc["S"]),
            "recip_d": pc["recip"],
            "w_pre": np.asarray(w_pre), "w_conv": np.asarray(w_conv),
            "w_dist": np.asarray(w_dist), "w_d0": np.asarray(w_d0),
            "w_fin": np.asarray(w_fin), "biases": biases,
        })

    res = run_bass_kernel_spmd(nc, in_maps, core_ids=list(range(NCORES)), trace=False)

    out = np.zeros(N, dtype=np.float32)
    for c in range(NCORES):
        smap = per_core[c]["slotmap"]
        valid = smap >= 0
        out[smap[valid]] = res.results[c]["out_d"][0][valid]
    return out


# revision 10
# speedup vs baseline: 1.0686x; 1.0686x over previous
"""AttributeDecoupledGNN Trainium2 kernel (8-core SPMD).

Strategy:
  - All node features kept transposed on-chip: [128 feats, node-slots].
  - Nodes dst-sharded: 12500/core, assigned to 13312 "slots" (208 bins x 64)
    via balanced bin-packing so each (bin, src-chunk) has <= 256 edges ->
    exactly 2 gather tiles of 128 edges -> cross-core-uniform program.
  - mean-aggregation = dma_gather (bf16 256B rows, int16 idx, 4 chunks of
    26624 table rows) + PE one-hot S-matmul (fp8 S) into PSUM windows of 512
    slots, accumulated chunk-by-chunk into an SBUF f32 accumulator, then
    scaled by 1/deg.
  - h shards exchanged between layers via AllGather collectives into a
    row-major gather table.
  - dist path + final layer folded: logits = h3 @ (W_np @ fW_a) +
    y3 @ (d_W3 @ fW_b) + const.
"""
import numpy as np
import ml_dtypes

import concourse.bass as bass
import concourse.bacc as bacc
import concourse.tile as tile
import concourse.mybir as mybir
from concourse.bass_utils import run_bass_kernel_spmd
from concourse.masks import make_identity

dt = mybir.dt
P = 128

# ---------------- problem constants (hardcoded) ----------------
N = 100000
E = 1600000
F_IN = 256
H = 128
KATT = 5
NCORES = 8
NSH = N // NCORES              # 12500
SLOTS = 13312                  # 26 windows * 512 = 208 bins * 64
WINDOWS = SLOTS // 512         # 26
BINS = SLOTS // 64             # 208
BIN_COLS = 64
T_S = 2                        # tiles per (bin, chunk)
NCHUNKS = 4
CHUNK_ROWS = 2 * SLOTS         # 26624
TILES_PER_CHUNK = BINS * T_S   # 416
IDX_PER_CHUNK = TILES_PER_CHUNK * 128   # 53248
CALL_TILES = 32                # 2 windows per gather call
CALLS_PER_CHUNK = (TILES_PER_CHUNK + CALL_TILES - 1) // CALL_TILES  # 13
NTAB = NCORES * SLOTS          # 106496
NODE_CHUNK = 512               # nodes per dense-phase matmul


# ================= host preprocessing =================

def _wrap_idx(idxs):
    return idxs.reshape(-1, 16).T.copy()


def _assign_bins(cnt):
    cap = T_S * 128
    fill = np.zeros((BINS, NCHUNKS), dtype=np.int64)
    ncols = np.zeros(BINS, dtype=np.int64)
    order = np.argsort(-cnt.max(axis=1), kind="stable")
    slot = np.full(cnt.shape[0], -1, dtype=np.int64)
    for d in order:
        c = cnt[d]
        new_fill = fill + c[None, :]
        feas = (new_fill <= cap).all(axis=1) & (ncols < BIN_COLS)
        if not feas.any():
            raise RuntimeError("bin packing infeasible")
        score = new_fill.max(axis=1).astype(np.float64)
        score[~feas] = np.inf
        b = int(np.argmin(score + 0.001 * ncols))
        slot[d] = b * BIN_COLS + ncols[b]
        ncols[b] += 1
        fill[b] += c
    return slot


def _preprocess_edges(edge_index):
    src = np.asarray(edge_index[0], dtype=np.int64)
    dst = np.asarray(edge_index[1], dtype=np.int64)

    deg = np.bincount(dst, minlength=N).astype(np.float32)
    recip_node = 1.0 / np.maximum(deg, 1.0)

    dst_owner = dst // NSH
    dst_local = dst % NSH
    src_owner = src // NSH
    chunk = src_owner // 2

    slot_of_node = np.zeros(N, dtype=np.int64)
    core_slotmap = []
    for c in range(NCORES):
        m = dst_owner == c
        cnt = np.zeros((NSH, NCHUNKS), dtype=np.int64)
        np.add.at(cnt, (dst_local[m], chunk[m]), 1)
        slot = _assign_bins(cnt)
        nodes = c * NSH + np.arange(NSH)
        slot_of_node[nodes] = slot
        smap = np.full(SLOTS, -1, dtype=np.int64)
        smap[slot] = nodes
        core_slotmap.append(smap)
    global_row_of_node = (np.arange(N) // NSH) * SLOTS + slot_of_node

    per_core = []
    for c in range(NCORES):
        m = dst_owner == c
        e_src_row = global_row_of_node[src[m]]
        e_slot = slot_of_node[dst[m]]
        e_chunk = e_src_row // CHUNK_ROWS
        e_idx_local = e_src_row % CHUNK_ROWS
        e_bin = e_slot // BIN_COLS
        e_col = e_slot % BIN_COLS

        key = e_chunk * BINS + e_bin
        order = np.argsort(key, kind="stable")
        key_s = key[order]
        idx_s = e_idx_local[order]
        col_s = e_col[order]
        bounds = np.searchsorted(key_s, np.arange(NCHUNKS * BINS + 1))

        idx_stream = np.zeros(NCHUNKS * IDX_PER_CHUNK, dtype=np.int16)
        scol_stream = np.full(NCHUNKS * IDX_PER_CHUNK, -1, dtype=np.int16)
        cap = T_S * 128
        for q in range(NCHUNKS):
            for b in range(BINS):
                k = q * BINS + b
                lo, hi = bounds[k], bounds[k + 1]
                n = hi - lo
                base = q * IDX_PER_CHUNK + b * cap
                idx_stream[base:base + n] = idx_s[lo:hi]
                scol_stream[base:base + n] = col_s[lo:hi]

        idx_wrapped = np.zeros((16, NCHUNKS * IDX_PER_CHUNK // 16), dtype=np.int16)
        off = 0
        for q in range(NCHUNKS):
            for k in range(CALLS_PER_CHUNK):
                t0 = k * CALL_TILES
                t1 = min(t0 + CALL_TILES, TILES_PER_CHUNK)
                nidx = (t1 - t0) * 128
                seg = idx_stream[q * IDX_PER_CHUNK + t0 * 128:
                                 q * IDX_PER_CHUNK + t1 * 128]
                idx_wrapped[:, off:off + nidx // 16] = _wrap_idx(seg)
                off += nidx // 16
        idx_rep = np.zeros((128, NCHUNKS * IDX_PER_CHUNK // 16), dtype=np.int16)
        for g in range(8):
            idx_rep[g * 16:(g + 1) * 16] = idx_wrapped

        ntiles = NCHUNKS * TILES_PER_CHUNK
        S = np.zeros((128, ntiles * BIN_COLS), dtype=ml_dtypes.float8_e4m3)
        scol_t = scol_stream.reshape(ntiles, 128)
        tt, pp = np.nonzero(scol_t >= 0)
        S[pp, tt * BIN_COLS + scol_t[tt, pp]] = 1.0

        smap = core_slotmap[c]
        recip_slot = np.zeros(SLOTS, dtype=np.float32)
        valid = smap >= 0
        recip_slot[valid] = recip_node[smap[valid]]

        per_core.append(dict(idx=idx_rep, S=S,
                             recip=np.broadcast_to(recip_slot[None, :],
                                                   (128, SLOTS)).copy(),
                             slotmap=smap))

    return per_core, global_row_of_node, slot_of_node


# ================= device program =================

def _build_program():
    nc = bacc.Bacc("TRN2", target_bir_lowering=False, debug=False,
                   enable_asserts=False, num_devices=NCORES)

    # per-core inputs
    x_t = nc.dram_tensor("x_t", [2, 128, SLOTS], dt.bfloat16, kind="ExternalInput")
    x_full = nc.dram_tensor("x_full", [2, 128, NTAB], dt.bfloat16, kind="ExternalInput")
    attr_t = nc.dram_tensor("attr_t", [KATT, SLOTS], dt.bfloat16, kind="ExternalInput")
    idx_d = nc.dram_tensor("idx_d", [128, NCHUNKS * IDX_PER_CHUNK // 16], dt.int16,
                           kind="ExternalInput")
    s_d = nc.dram_tensor("s_d", [128, NCHUNKS * TILES_PER_CHUNK * BIN_COLS],
                         dt.float8e4, kind="ExternalInput")
    recip_d = nc.dram_tensor("recip_d", [128, WINDOWS * 512], dt.float32, kind="ExternalInput")
    # replicated weights
    w_pre = nc.dram_tensor("w_pre", [2, 128, H], dt.bfloat16, kind="ExternalInput")
    w_conv = nc.dram_tensor("w_conv", [4, 128, H], dt.bfloat16, kind="ExternalInput")
    w_dist = nc.dram_tensor("w_dist", [2, 128, H], dt.bfloat16, kind="ExternalInput")
    w_d0 = nc.dram_tensor("w_d0", [KATT, H], dt.bfloat16, kind="ExternalInput")
    w_fin = nc.dram_tensor("w_fin", [2, 128, 1], dt.bfloat16, kind="ExternalInput")
    biases = nc.dram_tensor("biases", [128, 8], dt.float32, kind="ExternalInput")
    # biases cols: 0=pre_b 1=c1_b 2=c2_b 3=d_b0 4=d_b1 5=d_b2 6=(c0 scalar in [0,6]) 7=unused

    out_d = nc.dram_tensor("out_d", [1, SLOTS], dt.float32, kind="ExternalOutput")

    AF = mybir.ActivationFunctionType

    with tile.TileContext(nc) as tc:
        with (
            tc.tile_pool(name="res", bufs=1) as res,
            tc.tile_pool(name="sb", bufs=2) as sb,
            tc.tile_pool(name="ps", bufs=2, space="PSUM") as ps,
            tc.tile_pool(name="dram", bufs=1, space="DRAM") as dram,
        ):
            # ---- resident tiles ----
            h_cur = res.tile([128, SLOTS], dt.bfloat16, tag="h_a")    # h1/h3
            h_nxt = res.tile([128, SLOTS], dt.bfloat16, tag="h_b")    # h2
            agg_t = res.tile([128, SLOTS], dt.bfloat16, tag="agg")
            acc = res.tile([128, SLOTS], dt.float32, tag="acc")
            wpre_sb = res.tile([128, 2 * H], dt.bfloat16, tag="wpre")
            wconv_sb = res.tile([128, 4 * H], dt.bfloat16, tag="wconv")
            wdist_sb = res.tile([128, 2 * H], dt.bfloat16, tag="wdist")
            wd0_sb = res.tile([KATT, H], dt.bfloat16, tag="wd0")
            wfin_sb = res.tile([128, 2], dt.bfloat16, tag="wfin")
            bias_sb = res.tile([128, 8], dt.float32, tag="bias")
            ident = res.tile([128, 128], dt.bfloat16, tag="ident")

            nc.sync.dma_start(wpre_sb[:].rearrange("p (k h) -> p k h", k=2), w_pre.ap().rearrange("k p h -> p k h"))
            nc.sync.dma_start(wconv_sb[:].rearrange("p (k h) -> p k h", k=4), w_conv.ap().rearrange("k p h -> p k h"))
            nc.sync.dma_start(wdist_sb[:].rearrange("p (k h) -> p k h", k=2), w_dist.ap().rearrange("k p h -> p k h"))
            nc.sync.dma_start(wd0_sb[:], w_d0[:])
            nc.sync.dma_start(wfin_sb[:].rearrange("p (k o) -> p k o", k=2), w_fin.ap().rearrange("k p o -> p k o"))
            nc.sync.dma_start(bias_sb[:], biases[:])
            make_identity(nc, ident[:])

            # gather tables + exchange bounce (DRAM)
            table1s = [dram.tile([CHUNK_ROWS, H], dt.bfloat16,
                                 tag=f"table1_{q}", name=f"table1_{q}")
                       for q in range(NCHUNKS)]
            table2 = dram.tile([NTAB, H], dt.bfloat16, tag="table2", addr_space="Shared")
            bounce2 = dram.tile([SLOTS, H], dt.bfloat16, tag="bounce2")

            # ---------------- dense helpers ----------------

            def pre_full_phase():
                """full-graph pre-matmul -> row-major table1 (local, no collective)."""
                for j in range(NTAB // NODE_CHUNK):
                    js = slice(j * NODE_CHUNK, (j + 1) * NODE_CHUNK)
                    xs = sb.tile([128, 2, NODE_CHUNK], dt.bfloat16, tag="xstage")
                    nc.sync.dma_start(
                        xs[:], x_full.ap()[:, :, js].rearrange("k p n -> p k n"))
                    pm = ps.tile([128, NODE_CHUNK], dt.float32, space="PSUM", tag="mm")
                    nc.tensor.matmul(pm[:], lhsT=wpre_sb[:, 0:H], rhs=xs[:, 0, :],
                                     start=True, stop=False)
                    nc.tensor.matmul(pm[:], lhsT=wpre_sb[:, H:2 * H], rhs=xs[:, 1, :],
                                     start=False, stop=True)
                    hs = sb.tile([128, NODE_CHUNK], dt.bfloat16, tag="hstage")
                    nc.vector.tensor_add(
                        hs[:], in0=pm[:],
                        in1=bias_sb[:, 0:1].to_broadcast([128, NODE_CHUNK]))
                    rs = sb.tile([128, 4, 128], dt.bfloat16, tag="rowstage")
                    for b in range(4):
                        pt = ps.tile([128, 128], dt.bfloat16, space="PSUM", tag="tr")
                        nc.tensor.transpose(out=pt[:], in_=hs[:, b * 128:(b + 1) * 128],
                                            identity=ident[:])
                        nc.scalar.copy(rs[:, b, :], pt[:])
                    q = j // (NTAB // NODE_CHUNK // NCHUNKS)
                    jl = j % (NTAB // NODE_CHUNK // NCHUNKS)
                    nc.sync.dma_start(
                        table1s[q][jl * NODE_CHUNK:(jl + 1) * NODE_CHUNK, :]
                        .rearrange("(b p) d -> p b d", p=128),
                        rs[:])

            def pre_phase():
                """h_cur[:, :] = x @ pre_W + pre_b (sharded, transposed)."""
                for j in range(SLOTS // NODE_CHUNK):
                    js = slice(j * NODE_CHUNK, (j + 1) * NODE_CHUNK)
                    xs = sb.tile([128, 2, NODE_CHUNK], dt.bfloat16, tag="xstage")
                    nc.sync.dma_start(
                        xs[:], x_t.ap()[:, :, js].rearrange("k p n -> p k n"))
                    pm = ps.tile([128, NODE_CHUNK], dt.float32, space="PSUM", tag="mm")
                    nc.tensor.matmul(pm[:], lhsT=wpre_sb[:, 0:H], rhs=xs[:, 0, :],
                                     start=True, stop=False)
                    nc.tensor.matmul(pm[:], lhsT=wpre_sb[:, H:2 * H], rhs=xs[:, 1, :],
                                     start=False, stop=True)
                    nc.vector.tensor_add(
                        h_cur[:, js], in0=pm[:],
                        in1=bias_sb[:, 0:1].to_broadcast([128, NODE_CHUNK]))

            def conv_phase(h_in, h_out, w_off, bias_col):
                """h_out = relu(Ws.T h_in + Wn.T agg + b)."""
                for j in range(SLOTS // NODE_CHUNK):
                    js = slice(j * NODE_CHUNK, (j + 1) * NODE_CHUNK)
                    pm = ps.tile([128, NODE_CHUNK], dt.float32, space="PSUM", tag="mm")
                    nc.tensor.matmul(pm[:], lhsT=wconv_sb[:, w_off * H:(w_off + 1) * H],
                                     rhs=h_in[:, js], start=True, stop=False)
                    nc.tensor.matmul(pm[:], lhsT=wconv_sb[:, (w_off + 1) * H:(w_off + 2) * H],
                                     rhs=agg_t[:, js], start=False, stop=True)
                    nc.scalar.activation(h_out[:, js], pm[:], AF.Relu,
                                         bias=bias_sb[:, bias_col:bias_col + 1])

            def exchange(h_shard, bounce, table):
                """transpose shard -> bounce -> AllGather -> table."""
                for j in range(SLOTS // NODE_CHUNK):
                    rs = sb.tile([128, 4, 128], dt.bfloat16, tag="rowstage")
                    for b in range(4):
                        col = j * NODE_CHUNK + b * 128
                        pt = ps.tile([128, 128], dt.bfloat16, space="PSUM", tag="tr")
                        nc.tensor.transpose(out=pt[:], in_=h_shard[:, col:col + 128],
                                            identity=ident[:])
                        nc.scalar.copy(rs[:, b, :], pt[:])
                    nc.sync.dma_start(
                        bounce[j * NODE_CHUNK:(j + 1) * NODE_CHUNK, :]
                        .rearrange("(b p) d -> p b d", p=128),
                        rs[:])
                nc.gpsimd.collective_compute(
                    "AllGather", mybir.AluOpType.bypass,
                    replica_groups=[list(range(NCORES))],
                    ins=[bounce.opt()],
                    outs=[table.opt()],
                )

            def agg_phase(tables):
                """acc = segment-sum over edges (gather + S matmul); agg_t = acc * recip."""
                for q in range(NCHUNKS):
                    ih = sb.tile([128, IDX_PER_CHUNK // 16], dt.int16, tag="idxstage")
                    nc.sync.dma_start(
                        ih[:], idx_d[:, q * (IDX_PER_CHUNK // 16):
                                     (q + 1) * (IDX_PER_CHUNK // 16)])
                    SGRP = 32  # tiles per S stage (2 windows)
                    shs = []
                    for g in range(TILES_PER_CHUNK // SGRP):
                        sh = sb.tile([128, SGRP * BIN_COLS], dt.float8e4, tag="sstage")
                        base = (q * TILES_PER_CHUNK + g * SGRP) * BIN_COLS
                        nc.scalar.dma_start(
                            sh[:], s_d[:, base:base + SGRP * BIN_COLS])
                        shs.append(sh)

                    gts = []
                    for k in range(CALLS_PER_CHUNK):
                        t0 = k * CALL_TILES
                        t1 = min(t0 + CALL_TILES, TILES_PER_CHUNK)
                        nidx = (t1 - t0) * 128
                        gt = sb.tile([128, CALL_TILES, H], dt.bfloat16, tag="gbuf")
                        nc.gpsimd.dma_gather(
                            gt[:, 0:(t1 - t0), :],
                            tables[q],
                            ih[:, t0 * 8:t0 * 8 + nidx // 16],
                            nidx, nidx, H, single_packet=False,
                        )
                        gts.append((gt, t0, t1))

                    # consume: per window (8 bins = 16 tiles)
                    for w in range(WINDOWS):
                        pw = ps.tile([128, 512], dt.float32, space="PSUM", tag="aggps")
                        for bi in range(8):
                            b = w * 8 + bi
                            for s_i in range(T_S):
                                t = b * T_S + s_i
                                gt, t0, t1 = gts[t // CALL_TILES]
                                sg = t // 32
                                soff = (t - sg * 32) * BIN_COLS
                                nc.tensor.matmul(
                                    pw[:, bi * BIN_COLS:(bi + 1) * BIN_COLS],
                                    lhsT=gt[:, t - t0, :],
                                    rhs=shs[sg][:, soff:soff + BIN_COLS],
                                    start=(bi == 0 and s_i == 0),
                                    stop=(bi == 7 and s_i == T_S - 1),
                                )
                        ws = slice(w * 512, (w + 1) * 512)
                        if q == 0:
                            nc.scalar.copy(acc[:, ws], pw[:])
                        else:
                            nc.vector.tensor_add(acc[:, ws], in0=acc[:, ws], in1=pw[:])

                # scale by recip -> bf16 agg
                for w in range(WINDOWS):
                    ws = slice(w * 512, (w + 1) * 512)
                    rc = sb.tile([128, 512], dt.float32, tag="recip")
                    nc.sync.dma_start(rc[:], recip_d[:, w * 512:(w + 1) * 512])
                    nc.vector.tensor_mul(agg_t[:, ws], in0=acc[:, ws], in1=rc[:])

            def dist_final_phase(h3):
                """fused dist MLP + folded final layer + sigmoid."""
                for j in range(SLOTS // NODE_CHUNK):
                    js = slice(j * NODE_CHUNK, (j + 1) * NODE_CHUNK)
                    at = sb.tile([KATT, NODE_CHUNK], dt.bfloat16, tag="attrstage")
                    nc.sync.dma_start(at[:], attr_t.ap()[:, js])
                    p1 = ps.tile([128, NODE_CHUNK], dt.float32, space="PSUM", tag="mm")
                    nc.tensor.matmul(p1[:], lhsT=wd0_sb[:], rhs=at[:],
                                     start=True, stop=True)
                    y1 = sb.tile([128, NODE_CHUNK], dt.bfloat16, tag="y1")
                    nc.scalar.activation(y1[:], p1[:], AF.Relu, bias=bias_sb[:, 3:4])
                    p2 = ps.tile([128, NODE_CHUNK], dt.float32, space="PSUM", tag="mm")
                    nc.tensor.matmul(p2[:], lhsT=wdist_sb[:, 0:H], rhs=y1[:],
                                     start=True, stop=True)
                    y2 = sb.tile([128, NODE_CHUNK], dt.bfloat16, tag="y2")
                    nc.scalar.activation(y2[:], p2[:], AF.Relu, bias=bias_sb[:, 4:5])
                    p3 = ps.tile([128, NODE_CHUNK], dt.float32, space="PSUM", tag="mm")
                    nc.tensor.matmul(p3[:], lhsT=wdist_sb[:, H:2 * H], rhs=y2[:],
                                     start=True, stop=True)
                    y3 = sb.tile([128, NODE_CHUNK], dt.bfloat16, tag="y3")
                    nc.scalar.activation(y3[:], p3[:], AF.Relu, bias=bias_sb[:, 5:6])
                    pf = ps.tile([1, NODE_CHUNK], dt.float32, space="PSUM", tag="fin")
                    nc.tensor.matmul(pf[:], lhsT=wfin_sb[:, 0:1], rhs=h3[:, js],
                                     start=True, stop=False)
                    nc.tensor.matmul(pf[:], lhsT=wfin_sb[:, 1:2], rhs=y3[:],
                                     start=False, stop=True)
                    ot = sb.tile([1, NODE_CHUNK], dt.float32, tag="ostage")
                    nc.scalar.activation(ot[:], pf[:], AF.Sigmoid,
                                         bias=bias_sb[0:1, 6:7])
                    nc.sync.dma_start(out_d[:, js], ot[:])

            # ---------------- schedule ----------------
            pre_full_phase()                   # table1 = h1 (all rows, local)
            pre_phase()                        # h_cur = h1 own shard
            agg_phase([t[:] for t in table1s])  # agg_t = mean_agg(h1)
            conv_phase(h_cur, h_nxt, 0, 1)     # h_nxt = h2
            exchange(h_nxt, bounce2, table2)   # table2 = h2
            agg_phase([table2[q * CHUNK_ROWS:(q + 1) * CHUNK_ROWS, :]
                       for q in range(NCHUNKS)])  # agg_t = mean_agg(h2)
            conv_phase(h_nxt, h_cur, 2, 2)     # h_cur = h3
            dist_final_phase(h_cur)

    nc.compile()
    return nc


_PROGRAM_CACHE = {}


def kernel(**inputs):
    x = np.asarray(inputs["x"], dtype=np.float32)
    edge_index = np.asarray(inputs["edge_index"])
    edge_attr = np.asarray(inputs["edge_attr"], dtype=np.float32)

    per_core, global_row_of_node, slot_of_node = _preprocess_edges(edge_index)

    bf = ml_dtypes.bfloat16
    f32 = np.float32

    pre_W = np.asarray(inputs["pre_W"], f32)
    w_pre = np.ascontiguousarray(pre_W.reshape(2, 128, H)).astype(bf)
    w_conv = np.stack([np.asarray(inputs["c1_Ws"], f32), np.asarray(inputs["c1_Wn"], f32),
                       np.asarray(inputs["c2_Ws"], f32), np.asarray(inputs["c2_Wn"], f32)]
                      ).astype(bf)
    w_dist = np.stack([np.asarray(inputs["d_W1"], f32),
                       np.asarray(inputs["d_W2"], f32)]).astype(bf)
    w_d0 = np.asarray(inputs["d_W0"], f32).astype(bf)

    fW = np.asarray(inputs["final_W"], f32)           # [256, 1]
    w1 = np.asarray(inputs["nodepost_W"], f32) @ fW[:128]   # [128,1]
    w2 = np.asarray(inputs["d_W3"], f32) @ fW[128:]         # [128,1]
    w_fin = np.stack([w1, w2]).astype(bf)                   # [2,128,1]
    c0 = float(np.asarray(inputs["nodepost_b"], f32) @ fW[:128, 0]
               + np.asarray(inputs["d_b3"], f32) @ fW[128:, 0]
               + np.asarray(inputs["final_b"], f32)[0])

    biases = np.zeros((128, 8), f32)
    biases[:, 0] = np.asarray(inputs["pre_b"], f32)
    biases[:, 1] = np.asarray(inputs["c1_b"], f32)
    biases[:, 2] = np.asarray(inputs["c2_b"], f32)
    biases[:, 3] = np.asarray(inputs["d_b0"], f32)
    biases[:, 4] = np.asarray(inputs["d_b1"], f32)
    biases[:, 5] = np.asarray(inputs["d_b2"], f32)
    biases[0, 6] = c0

    if "nc" not in _PROGRAM_CACHE:
        _PROGRAM_CACHE["nc"] = _build_program()
    nc = _PROGRAM_CACHE["nc"]

    x_ts = []
    for c in range(NCORES):
        smap = per_core[c]["slotmap"]
        valid = smap >= 0
        x_tc = np.zeros((2, 128, SLOTS), bf)
        xv = x[smap[valid]].astype(bf)                 # [n_valid, 256]
        x_tc[:, :, :][..., valid] = xv.T.reshape(2, 128, -1)
        x_ts.append(x_tc)
    x_full_np = np.concatenate(x_ts, axis=2)           # [2, 128, NTAB]

    in_maps = []
    for c in range(NCORES):
        pc = per_core[c]
        smap = pc["slotmap"]
        valid = smap >= 0
        attr_t = np.zeros((KATT, SLOTS), bf)
        attr_t[:, valid] = np.asarray(edge_attr, f32)[smap[valid]].T.astype(bf)
        in_maps.append({
            "x_t": x_ts[c], "x_full": x_full_np, "attr_t": attr_t,
            "idx_d": pc["idx"], "s_d": np.asarray(pc["S"]),
            "recip_d": pc["recip"],
            "w_pre": np.asarray(w_pre), "w_conv": np.asarray(w_conv),
            "w_dist": np.asarray(w_dist), "w_d0": np.asarray(w_d0),
            "w_fin": np.asarray(w_fin), "biases": biases,
        })

    res = run_bass_kernel_spmd(nc, in_maps, core_ids=list(range(NCORES)), trace=False)

    out = np.zeros(N, dtype=np.float32)
    for c in range(NCORES):
        smap = per_core[c]["slotmap"]
        valid = smap >= 0
        out[smap[valid]] = res.results[c]["out_d"][0][valid]
    return out


# revision 11
# speedup vs baseline: 1.1025x; 1.0317x over previous
"""AttributeDecoupledGNN Trainium2 kernel (8-core SPMD).

Strategy:
  - All node features kept transposed on-chip: [128 feats, node-slots].
  - Nodes dst-sharded: 12500/core, assigned to 13312 "slots" (208 bins x 64)
    via balanced bin-packing so each (bin, src-chunk) has <= 256 edges ->
    exactly 2 gather tiles of 128 edges -> cross-core-uniform program.
  - mean-aggregation = dma_gather (bf16 256B rows, int16 idx, 4 chunks of
    26624 table rows) + PE one-hot S-matmul (fp8 S) into PSUM windows of 512
    slots, accumulated chunk-by-chunk into an SBUF f32 accumulator, then
    scaled by 1/deg.
  - h shards exchanged between layers via AllGather collectives into a
    row-major gather table.
  - dist path + final layer folded: logits = h3 @ (W_np @ fW_a) +
    y3 @ (d_W3 @ fW_b) + const.
"""
import numpy as np
import ml_dtypes

import concourse.bass as bass
import concourse.bacc as bacc
import concourse.tile as tile
import concourse.mybir as mybir
from concourse.bass_utils import run_bass_kernel_spmd
from concourse.masks import make_identity

dt = mybir.dt
P = 128

# ---------------- problem constants (hardcoded) ----------------
N = 100000
E = 1600000
F_IN = 256
H = 128
KATT = 5
NCORES = 8
NSH = N // NCORES              # 12500
SLOTS = 13312                  # 26 windows * 512 = 208 bins * 64
WINDOWS = SLOTS // 512         # 26
BINS = SLOTS // 64             # 208
BIN_COLS = 64
T_S = 2                        # tiles per (bin, chunk)
NCHUNKS = 4
CHUNK_ROWS = 2 * SLOTS         # 26624
TILES_PER_CHUNK = BINS * T_S   # 416
IDX_PER_CHUNK = TILES_PER_CHUNK * 128   # 53248
CALL_TILES = 52                # tiles per gather call (8 calls/chunk)
CALLS_PER_CHUNK = (TILES_PER_CHUNK + CALL_TILES - 1) // CALL_TILES  # 8
NTAB = NCORES * SLOTS          # 106496
NODE_CHUNK = 512               # nodes per dense-phase matmul


# ================= host preprocessing =================

def _wrap_idx(idxs):
    return idxs.reshape(-1, 16).T.copy()


def _assign_bins(cnt):
    cap = T_S * 128
    fill = np.zeros((BINS, NCHUNKS), dtype=np.int64)
    ncols = np.zeros(BINS, dtype=np.int64)
    order = np.argsort(-cnt.max(axis=1), kind="stable")
    slot = np.full(cnt.shape[0], -1, dtype=np.int64)
    for d in order:
        c = cnt[d]
        new_fill = fill + c[None, :]
        feas = (new_fill <= cap).all(axis=1) & (ncols < BIN_COLS)
        if not feas.any():
            raise RuntimeError("bin packing infeasible")
        score = new_fill.max(axis=1).astype(np.float64)
        score[~feas] = np.inf
        b = int(np.argmin(score + 0.001 * ncols))
        slot[d] = b * BIN_COLS + ncols[b]
        ncols[b] += 1
        fill[b] += c
    return slot


def _preprocess_edges(edge_index):
    src = np.asarray(edge_index[0], dtype=np.int64)
    dst = np.asarray(edge_index[1], dtype=np.int64)

    deg = np.bincount(dst, minlength=N).astype(np.float32)
    recip_node = 1.0 / np.maximum(deg, 1.0)

    dst_owner = dst // NSH
    dst_local = dst % NSH
    src_owner = src // NSH
    chunk = src_owner // 2

    slot_of_node = np.zeros(N, dtype=np.int64)
    core_slotmap = []
    for c in range(NCORES):
        m = dst_owner == c
        cnt = np.zeros((NSH, NCHUNKS), dtype=np.int64)
        np.add.at(cnt, (dst_local[m], chunk[m]), 1)
        slot = _assign_bins(cnt)
        nodes = c * NSH + np.arange(NSH)
        slot_of_node[nodes] = slot
        smap = np.full(SLOTS, -1, dtype=np.int64)
        smap[slot] = nodes
        core_slotmap.append(smap)
    global_row_of_node = (np.arange(N) // NSH) * SLOTS + slot_of_node

    per_core = []
    for c in range(NCORES):
        m = dst_owner == c
        e_src_row = global_row_of_node[src[m]]
        e_slot = slot_of_node[dst[m]]
        e_chunk = e_src_row // CHUNK_ROWS
        e_idx_local = e_src_row % CHUNK_ROWS
        e_bin = e_slot // BIN_COLS
        e_col = e_slot % BIN_COLS

        key = e_chunk * BINS + e_bin
        order = np.argsort(key, kind="stable")
        key_s = key[order]
        idx_s = e_idx_local[order]
        col_s = e_col[order]
        bounds = np.searchsorted(key_s, np.arange(NCHUNKS * BINS + 1))

        idx_stream = np.zeros(NCHUNKS * IDX_PER_CHUNK, dtype=np.int16)
        scol_stream = np.full(NCHUNKS * IDX_PER_CHUNK, -1, dtype=np.int16)
        cap = T_S * 128
        for q in range(NCHUNKS):
            for b in range(BINS):
                k = q * BINS + b
                lo, hi = bounds[k], bounds[k + 1]
                n = hi - lo
                base = q * IDX_PER_CHUNK + b * cap
                idx_stream[base:base + n] = idx_s[lo:hi]
                scol_stream[base:base + n] = col_s[lo:hi]

        idx_wrapped = np.zeros((16, NCHUNKS * IDX_PER_CHUNK // 16), dtype=np.int16)
        off = 0
        for q in range(NCHUNKS):
            for k in range(CALLS_PER_CHUNK):
                t0 = k * CALL_TILES
                t1 = min(t0 + CALL_TILES, TILES_PER_CHUNK)
                nidx = (t1 - t0) * 128
                seg = idx_stream[q * IDX_PER_CHUNK + t0 * 128:
                                 q * IDX_PER_CHUNK + t1 * 128]
                idx_wrapped[:, off:off + nidx // 16] = _wrap_idx(seg)
                off += nidx // 16
        idx_rep = np.zeros((128, NCHUNKS * IDX_PER_CHUNK // 16), dtype=np.int16)
        for g in range(8):
            idx_rep[g * 16:(g + 1) * 16] = idx_wrapped

        ntiles = NCHUNKS * TILES_PER_CHUNK
        S = np.zeros((128, ntiles * BIN_COLS), dtype=ml_dtypes.float8_e4m3)
        scol_t = scol_stream.reshape(ntiles, 128)
        tt, pp = np.nonzero(scol_t >= 0)
        S[pp, tt * BIN_COLS + scol_t[tt, pp]] = 1.0

        smap = core_slotmap[c]
        recip_slot = np.zeros(SLOTS, dtype=np.float32)
        valid = smap >= 0
        recip_slot[valid] = recip_node[smap[valid]]

        per_core.append(dict(idx=idx_rep, S=S,
                             recip=np.broadcast_to(recip_slot[None, :],
                                                   (128, SLOTS)).copy(),
                             slotmap=smap))

    return per_core, global_row_of_node, slot_of_node


# ================= device program =================

def _build_program():
    nc = bacc.Bacc("TRN2", target_bir_lowering=False, debug=False,
                   enable_asserts=False, num_devices=NCORES)

    # per-core inputs
    x_t = nc.dram_tensor("x_t", [2, 128, SLOTS], dt.bfloat16, kind="ExternalInput")
    x_full = nc.dram_tensor("x_full", [2, 128, NTAB], dt.bfloat16, kind="ExternalInput")
    attr_t = nc.dram_tensor("attr_t", [KATT, SLOTS], dt.bfloat16, kind="ExternalInput")
    idx_d = nc.dram_tensor("idx_d", [128, NCHUNKS * IDX_PER_CHUNK // 16], dt.int16,
                           kind="ExternalInput")
    s_d = nc.dram_tensor("s_d", [128, NCHUNKS * TILES_PER_CHUNK * BIN_COLS],
                         dt.float8e4, kind="ExternalInput")
    recip_d = nc.dram_tensor("recip_d", [128, WINDOWS * 512], dt.float32, kind="ExternalInput")
    # replicated weights
    w_pre = nc.dram_tensor("w_pre", [2, 128, H], dt.bfloat16, kind="ExternalInput")
    w_conv = nc.dram_tensor("w_conv", [4, 128, H], dt.bfloat16, kind="ExternalInput")
    w_dist = nc.dram_tensor("w_dist", [2, 128, H], dt.bfloat16, kind="ExternalInput")
    w_d0 = nc.dram_tensor("w_d0", [KATT, H], dt.bfloat16, kind="ExternalInput")
    w_fin = nc.dram_tensor("w_fin", [2, 128, 1], dt.bfloat16, kind="ExternalInput")
    biases = nc.dram_tensor("biases", [128, 8], dt.float32, kind="ExternalInput")
    # biases cols: 0=pre_b 1=c1_b 2=c2_b 3=d_b0 4=d_b1 5=d_b2 6=(c0 scalar in [0,6]) 7=unused

    out_d = nc.dram_tensor("out_d", [1, SLOTS], dt.float32, kind="ExternalOutput")

    AF = mybir.ActivationFunctionType

    with tile.TileContext(nc) as tc:
        with (
            tc.tile_pool(name="res", bufs=1) as res,
            tc.tile_pool(name="sb", bufs=2) as sb,
            tc.tile_pool(name="ps", bufs=2, space="PSUM") as ps,
            tc.tile_pool(name="dram", bufs=1, space="DRAM") as dram,
        ):
            # ---- resident tiles ----
            h_cur = res.tile([128, SLOTS], dt.bfloat16, tag="h_a")    # h1/h3
            h_nxt = res.tile([128, SLOTS], dt.bfloat16, tag="h_b")    # h2
            agg_t = res.tile([128, SLOTS], dt.bfloat16, tag="agg")
            acc = res.tile([128, SLOTS], dt.float32, tag="acc")
            wpre_sb = res.tile([128, 2 * H], dt.bfloat16, tag="wpre")
            wconv_sb = res.tile([128, 4 * H], dt.bfloat16, tag="wconv")
            wdist_sb = res.tile([128, 2 * H], dt.bfloat16, tag="wdist")
            wd0_sb = res.tile([KATT, H], dt.bfloat16, tag="wd0")
            wfin_sb = res.tile([128, 2], dt.bfloat16, tag="wfin")
            bias_sb = res.tile([128, 8], dt.float32, tag="bias")
            ident = res.tile([128, 128], dt.bfloat16, tag="ident")

            nc.sync.dma_start(wpre_sb[:].rearrange("p (k h) -> p k h", k=2), w_pre.ap().rearrange("k p h -> p k h"))
            nc.sync.dma_start(wconv_sb[:].rearrange("p (k h) -> p k h", k=4), w_conv.ap().rearrange("k p h -> p k h"))
            nc.sync.dma_start(wdist_sb[:].rearrange("p (k h) -> p k h", k=2), w_dist.ap().rearrange("k p h -> p k h"))
            nc.sync.dma_start(wd0_sb[:], w_d0[:])
            nc.sync.dma_start(wfin_sb[:].rearrange("p (k o) -> p k o", k=2), w_fin.ap().rearrange("k p o -> p k o"))
            nc.sync.dma_start(bias_sb[:], biases[:])
            make_identity(nc, ident[:])

            # gather tables + exchange bounce (DRAM)
            table1s = [dram.tile([CHUNK_ROWS, H], dt.bfloat16,
                                 tag=f"table1_{q}", name=f"table1_{q}")
                       for q in range(NCHUNKS)]
            table2 = dram.tile([NTAB, H], dt.bfloat16, tag="table2", addr_space="Shared")
            bounce2 = dram.tile([SLOTS, H], dt.bfloat16, tag="bounce2")

            # ---------------- dense helpers ----------------

            def pre_full_phase():
                """full-graph pre-matmul -> row-major table1 (local, no collective)."""
                for j in range(NTAB // NODE_CHUNK):
                    js = slice(j * NODE_CHUNK, (j + 1) * NODE_CHUNK)
                    xs = sb.tile([128, 2, NODE_CHUNK], dt.bfloat16, tag="xstage")
                    nc.sync.dma_start(
                        xs[:], x_full.ap()[:, :, js].rearrange("k p n -> p k n"))
                    pm = ps.tile([128, NODE_CHUNK], dt.float32, space="PSUM", tag="mm")
                    nc.tensor.matmul(pm[:], lhsT=wpre_sb[:, 0:H], rhs=xs[:, 0, :],
                                     start=True, stop=False)
                    nc.tensor.matmul(pm[:], lhsT=wpre_sb[:, H:2 * H], rhs=xs[:, 1, :],
                                     start=False, stop=True)
                    hs = sb.tile([128, NODE_CHUNK], dt.bfloat16, tag="hstage")
                    nc.vector.tensor_add(
                        hs[:], in0=pm[:],
                        in1=bias_sb[:, 0:1].to_broadcast([128, NODE_CHUNK]))
                    rs = sb.tile([128, 4, 128], dt.bfloat16, tag="rowstage")
                    for b in range(4):
                        pt = ps.tile([128, 128], dt.bfloat16, space="PSUM", tag="tr")
                        nc.tensor.transpose(out=pt[:], in_=hs[:, b * 128:(b + 1) * 128],
                                            identity=ident[:])
                        nc.scalar.copy(rs[:, b, :], pt[:])
                    q = j // (NTAB // NODE_CHUNK // NCHUNKS)
                    jl = j % (NTAB // NODE_CHUNK // NCHUNKS)
                    nc.sync.dma_start(
                        table1s[q][jl * NODE_CHUNK:(jl + 1) * NODE_CHUNK, :]
                        .rearrange("(b p) d -> p b d", p=128),
                        rs[:])

            def pre_phase():
                """h_cur[:, :] = x @ pre_W + pre_b (sharded, transposed)."""
                for j in range(SLOTS // NODE_CHUNK):
                    js = slice(j * NODE_CHUNK, (j + 1) * NODE_CHUNK)
                    xs = sb.tile([128, 2, NODE_CHUNK], dt.bfloat16, tag="xstage")
                    nc.sync.dma_start(
                        xs[:], x_t.ap()[:, :, js].rearrange("k p n -> p k n"))
                    pm = ps.tile([128, NODE_CHUNK], dt.float32, space="PSUM", tag="mm")
                    nc.tensor.matmul(pm[:], lhsT=wpre_sb[:, 0:H], rhs=xs[:, 0, :],
                                     start=True, stop=False)
                    nc.tensor.matmul(pm[:], lhsT=wpre_sb[:, H:2 * H], rhs=xs[:, 1, :],
                                     start=False, stop=True)
                    nc.vector.tensor_add(
                        h_cur[:, js], in0=pm[:],
                        in1=bias_sb[:, 0:1].to_broadcast([128, NODE_CHUNK]))

            def conv_phase(h_in, h_out, w_off, bias_col):
                """h_out = relu(Ws.T h_in + Wn.T agg + b)."""
                for j in range(SLOTS // NODE_CHUNK):
                    js = slice(j * NODE_CHUNK, (j + 1) * NODE_CHUNK)
                    pm = ps.tile([128, NODE_CHUNK], dt.float32, space="PSUM", tag="mm")
                    nc.tensor.matmul(pm[:], lhsT=wconv_sb[:, w_off * H:(w_off + 1) * H],
                                     rhs=h_in[:, js], start=True, stop=False)
                    nc.tensor.matmul(pm[:], lhsT=wconv_sb[:, (w_off + 1) * H:(w_off + 2) * H],
                                     rhs=agg_t[:, js], start=False, stop=True)
                    nc.scalar.activation(h_out[:, js], pm[:], AF.Relu,
                                         bias=bias_sb[:, bias_col:bias_col + 1])

            def exchange(h_shard, bounce, table):
                """transpose shard -> bounce -> AllGather -> table."""
                for j in range(SLOTS // NODE_CHUNK):
                    rs = sb.tile([128, 4, 128], dt.bfloat16, tag="rowstage")
                    for b in range(4):
                        col = j * NODE_CHUNK + b * 128
                        pt = ps.tile([128, 128], dt.bfloat16, space="PSUM", tag="tr")
                        nc.tensor.transpose(out=pt[:], in_=h_shard[:, col:col + 128],
                                            identity=ident[:])
                        nc.scalar.copy(rs[:, b, :], pt[:])
                    nc.sync.dma_start(
                        bounce[j * NODE_CHUNK:(j + 1) * NODE_CHUNK, :]
                        .rearrange("(b p) d -> p b d", p=128),
                        rs[:])
                nc.gpsimd.collective_compute(
                    "AllGather", mybir.AluOpType.bypass,
                    replica_groups=[list(range(NCORES))],
                    ins=[bounce.opt()],
                    outs=[table.opt()],
                )

            def agg_phase(tables):
                """acc = segment-sum over edges (gather + S matmul); agg_t = acc * recip."""
                for q in range(NCHUNKS):
                    ih = sb.tile([128, IDX_PER_CHUNK // 16], dt.int16, tag="idxstage")
                    nc.sync.dma_start(
                        ih[:], idx_d[:, q * (IDX_PER_CHUNK // 16):
                                     (q + 1) * (IDX_PER_CHUNK // 16)])
                    SGRP = 32  # tiles per S stage (2 windows)
                    shs = []
                    for g in range(TILES_PER_CHUNK // SGRP):
                        sh = sb.tile([128, SGRP * BIN_COLS], dt.float8e4, tag="sstage")
                        base = (q * TILES_PER_CHUNK + g * SGRP) * BIN_COLS
                        nc.scalar.dma_start(
                            sh[:], s_d[:, base:base + SGRP * BIN_COLS])
                        shs.append(sh)

                    gts = []
                    for k in range(CALLS_PER_CHUNK):
                        t0 = k * CALL_TILES
                        t1 = min(t0 + CALL_TILES, TILES_PER_CHUNK)
                        nidx = (t1 - t0) * 128
                        gt = sb.tile([128, CALL_TILES, H], dt.bfloat16, tag="gbuf")
                        nc.gpsimd.dma_gather(
                            gt[:, 0:(t1 - t0), :],
                            tables[q],
                            ih[:, t0 * 8:t0 * 8 + nidx // 16],
                            nidx, nidx, H, single_packet=False,
                        )
                        gts.append((gt, t0, t1))

                    # consume: per window (8 bins = 16 tiles)
                    for w in range(WINDOWS):
                        pw = ps.tile([128, 512], dt.float32, space="PSUM", tag="aggps")
                        for bi in range(8):
                            b = w * 8 + bi
                            for s_i in range(T_S):
                                t = b * T_S + s_i
                                gt, t0, t1 = gts[t // CALL_TILES]
                                sg = t // 32
                                soff = (t - sg * 32) * BIN_COLS
                                nc.tensor.matmul(
                                    pw[:, bi * BIN_COLS:(bi + 1) * BIN_COLS],
                                    lhsT=gt[:, t - t0, :],
                                    rhs=shs[sg][:, soff:soff + BIN_COLS],
                                    start=(bi == 0 and s_i == 0),
                                    stop=(bi == 7 and s_i == T_S - 1),
                                )
                        ws = slice(w * 512, (w + 1) * 512)
                        if q == 0:
                            nc.scalar.copy(acc[:, ws], pw[:])
                        else:
                            nc.vector.tensor_add(acc[:, ws], in0=acc[:, ws], in1=pw[:])

                # scale by recip -> bf16 agg
                for w in range(WINDOWS):
                    ws = slice(w * 512, (w + 1) * 512)
                    rc = sb.tile([128, 512], dt.float32, tag="recip")
                    nc.sync.dma_start(rc[:], recip_d[:, w * 512:(w + 1) * 512])
                    nc.vector.tensor_mul(agg_t[:, ws], in0=acc[:, ws], in1=rc[:])

            def dist_final_phase(h3):
                """fused dist MLP + folded final layer + sigmoid."""
                for j in range(SLOTS // NODE_CHUNK):
                    js = slice(j * NODE_CHUNK, (j + 1) * NODE_CHUNK)
                    at = sb.tile([KATT, NODE_CHUNK], dt.bfloat16, tag="attrstage")
                    nc.sync.dma_start(at[:], attr_t.ap()[:, js])
                    p1 = ps.tile([128, NODE_CHUNK], dt.float32, space="PSUM", tag="mm")
                    nc.tensor.matmul(p1[:], lhsT=wd0_sb[:], rhs=at[:],
                                     start=True, stop=True)
                    y1 = sb.tile([128, NODE_CHUNK], dt.bfloat16, tag="y1")
                    nc.scalar.activation(y1[:], p1[:], AF.Relu, bias=bias_sb[:, 3:4])
                    p2 = ps.tile([128, NODE_CHUNK], dt.float32, space="PSUM", tag="mm")
                    nc.tensor.matmul(p2[:], lhsT=wdist_sb[:, 0:H], rhs=y1[:],
                                     start=True, stop=True)
                    y2 = sb.tile([128, NODE_CHUNK], dt.bfloat16, tag="y2")
                    nc.scalar.activation(y2[:], p2[:], AF.Relu, bias=bias_sb[:, 4:5])
                    p3 = ps.tile([128, NODE_CHUNK], dt.float32, space="PSUM", tag="mm")
                    nc.tensor.matmul(p3[:], lhsT=wdist_sb[:, H:2 * H], rhs=y2[:],
                                     start=True, stop=True)
                    y3 = sb.tile([128, NODE_CHUNK], dt.bfloat16, tag="y3")
                    nc.scalar.activation(y3[:], p3[:], AF.Relu, bias=bias_sb[:, 5:6])
                    pf = ps.tile([1, NODE_CHUNK], dt.float32, space="PSUM", tag="fin")
                    nc.tensor.matmul(pf[:], lhsT=wfin_sb[:, 0:1], rhs=h3[:, js],
                                     start=True, stop=False)
                    nc.tensor.matmul(pf[:], lhsT=wfin_sb[:, 1:2], rhs=y3[:],
                                     start=False, stop=True)
                    ot = sb.tile([1, NODE_CHUNK], dt.float32, tag="ostage")
                    nc.scalar.activation(ot[:], pf[:], AF.Sigmoid,
                                         bias=bias_sb[0:1, 6:7])
                    nc.sync.dma_start(out_d[:, js], ot[:])

            # ---------------- schedule ----------------
            pre_full_phase()                   # table1 = h1 (all rows, local)
            pre_phase()                        # h_cur = h1 own shard
            agg_phase([t[:] for t in table1s])  # agg_t = mean_agg(h1)
            conv_phase(h_cur, h_nxt, 0, 1)     # h_nxt = h2
            exchange(h_nxt, bounce2, table2)   # table2 = h2
            agg_phase([table2[q * CHUNK_ROWS:(q + 1) * CHUNK_ROWS, :]
                       for q in range(NCHUNKS)])  # agg_t = mean_agg(h2)
            conv_phase(h_nxt, h_cur, 2, 2)     # h_cur = h3
            dist_final_phase(h_cur)

    nc.compile()
    return nc


_PROGRAM_CACHE = {}


def kernel(**inputs):
    x = np.asarray(inputs["x"], dtype=np.float32)
    edge_index = np.asarray(inputs["edge_index"])
    edge_attr = np.asarray(inputs["edge_attr"], dtype=np.float32)

    per_core, global_row_of_node, slot_of_node = _preprocess_edges(edge_index)

    bf = ml_dtypes.bfloat16
    f32 = np.float32

    pre_W = np.asarray(inputs["pre_W"], f32)
    w_pre = np.ascontiguousarray(pre_W.reshape(2, 128, H)).astype(bf)
    w_conv = np.stack([np.asarray(inputs["c1_Ws"], f32), np.asarray(inputs["c1_Wn"], f32),
                       np.asarray(inputs["c2_Ws"], f32), np.asarray(inputs["c2_Wn"], f32)]
                      ).astype(bf)
    w_dist = np.stack([np.asarray(inputs["d_W1"], f32),
                       np.asarray(inputs["d_W2"], f32)]).astype(bf)
    w_d0 = np.asarray(inputs["d_W0"], f32).astype(bf)

    fW = np.asarray(inputs["final_W"], f32)           # [256, 1]
    w1 = np.asarray(inputs["nodepost_W"], f32) @ fW[:128]   # [128,1]
    w2 = np.asarray(inputs["d_W3"], f32) @ fW[128:]         # [128,1]
    w_fin = np.stack([w1, w2]).astype(bf)                   # [2,128,1]
    c0 = float(np.asarray(inputs["nodepost_b"], f32) @ fW[:128, 0]
               + np.asarray(inputs["d_b3"], f32) @ fW[128:, 0]
               + np.asarray(inputs["final_b"], f32)[0])

    biases = np.zeros((128, 8), f32)
    biases[:, 0] = np.asarray(inputs["pre_b"], f32)
    biases[:, 1] = np.asarray(inputs["c1_b"], f32)
    biases[:, 2] = np.asarray(inputs["c2_b"], f32)
    biases[:, 3] = np.asarray(inputs["d_b0"], f32)
    biases[:, 4] = np.asarray(inputs["d_b1"], f32)
    biases[:, 5] = np.asarray(inputs["d_b2"], f32)
    biases[0, 6] = c0

    if "nc" not in _PROGRAM_CACHE:
        _PROGRAM_CACHE["nc"] = _build_program()
    nc = _PROGRAM_CACHE["nc"]

    x_ts = []
    for c in range(NCORES):
        smap = per_core[c]["slotmap"]
        valid = smap >= 0
        x_tc = np.zeros((2, 128, SLOTS), bf)
        xv = x[smap[valid]].astype(bf)                 # [n_valid, 256]
        x_tc[:, :, :][..., valid] = xv.T.reshape(2, 128, -1)
        x_ts.append(x_tc)
    x_full_np = np.concatenate(x_ts, axis=2)           # [2, 128, NTAB]

    in_maps = []
    for c in range(NCORES):
        pc = per_core[c]
        smap = pc["slotmap"]
        valid = smap >= 0
        attr_t = np.zeros((KATT, SLOTS), bf)
        attr_t[:, valid] = np.asarray(edge_attr, f32)[smap[valid]].T.astype(bf)
        in_maps.append({
            "x_t": x_ts[c], "x_full": x_full_np, "attr_t": attr_t,
            "idx_d": pc["idx"], "s_d": np.asarray(pc["S"]),
            "recip_d": pc["recip"],
            "w_pre": np.asarray(w_pre), "w_conv": np.asarray(w_conv),
            "w_dist": np.asarray(w_dist), "w_d0": np.asarray(w_d0),
            "w_fin": np.asarray(w_fin), "biases": biases,
        })

    res = run_bass_kernel_spmd(nc, in_maps, core_ids=list(range(NCORES)), trace=False)

    out = np.zeros(N, dtype=np.float32)
    for c in range(NCORES):
        smap = per_core[c]["slotmap"]
        valid = smap >= 0
        out[smap[valid]] = res.results[c]["out_d"][0][valid]
    return out
